# revision 1
# baseline (speedup 1.0000x reference)
"""Trainium2 Bass kernel for nn_Attention_34840774705279 (sparse/deformable attention).

Math (matches reference.py):
  v   = x @ v_w.T + v_b            -> per-head maps [B*NH, H, W, HD]
  off = x @ off_w.T + off_b        -> off_w is structurally zero, so offsets are
                                      CONSTANT per (head, point); for this problem
                                      they are (+-p or ~1e-16) => integer shifts.
  w   = softmax_p(x @ aw_w.T + aw_b)
  out[i,j] = sum_p w_p[i,j] * v[i+dy_p, j+dx_p]   (zero outside the map)
  y   = out @ proj_w.T + proj_b

Sharding (8 cores, uniform SPMD program):
  core d -> batch b = d//2, row-half r0 = 64*(d%2). Each core computes ALL 8
  heads for its 64 output rows (8192 tokens) using a 4-row halo of v rows
  (host zero-pads x rows outside the image), so shifts up to +-4 never cross
  cores and no cross-core reduction is needed; the host just concatenates.

Device algorithm (per core):
  A. v+logit projection, pixel-major: host supplies x TRANSPOSED [256, 9216];
     per image row r: a[j, 0:288] = xT_row_r.T @ [v_w.T | aw_w.T] lands
     [j=col, (9 slots x 32)] in one PSUM bank and is copied once into the
     merged VL tile [j, 9, 72, 32] (slots 0-7 = per-head v maps, slot 8 =
     attention logits). All matmuls run in float32r (full PE rate at N>=256).
  B. softmax over the 4 points, batched across all 8 heads (exp on ScalarE,
     adds/reciprocal on VectorE), split by row-half into separate E tiles.
  C. sampling + weighting via weight-then-shift identity
        w .* (S_dx @ V_win) == S_dx @ ((S_-dx^T w) .* V_win):
     per (head, point): one tiny matmul computes the column-shifted weights
     E' = S_-dx^T E (alpha folded in), VectorE multiplies the 32-row V window
     (row shift dy = compile-time slot offset) by E' broadcast over d, and the
     0/1 column-shift matrix S_dx matmul-accumulates all 4 points directly in
     PSUM. Bilinear (fractional) offsets are supported as multiple corner
     terms; integer offsets (this problem) are a single term each.
  D. output projection: PE transposes OUT rows back to channel-major,
     y^T = proj.T @ OUT^T accumulated over both 128-channel halves; host
     transposes y^T back. proj_b applied via ScalarE bias (zero-bias inputs
     skip the bias matmuls entirely).
  Emission interleaves C-half0 into phase A's tail and D-half0 into C-half1
  so VectorE weighting overlaps PE/DMA work of neighbouring phases.
"""

import os
import sys
import math

import numpy as np

sys.path.insert(0, "/opt/trn_rl_repo")

P = 128
H = W = 128
NH, NP, HD = 8, 4, 32
DIM = 256
N_TOK = H * W
ROWS_OUT = 64          # output rows per core
HALO = 4
ROWS_V = ROWS_OUT + 2 * HALO   # 72 v-row slots per core
TOK_V = ROWS_V * W             # 9216
N_CORES = 8

F32R = os.environ.get("KERNEL_F32R", "1") == "1"

_cache = {}


def _build_terms(off_b):
    """Per (h, p): list of (dx, dy, alpha) corner terms from the constant offsets.

    General for any constant offset (bilinear corners); for this problem each
    (h, p) yields exactly one term with alpha ~= 1."""
    ob = np.asarray(off_b, np.float64).reshape(NH, NP, 2)
    terms = [[[] for _ in range(NP)] for _ in range(NH)]
    for h in range(NH):
        for p in range(NP):
            fx, fy = ob[h, p, 0], ob[h, p, 1]
            x0 = math.floor(fx)
            y0 = math.floor(fy)
            wx1 = fx - x0
            wy1 = fy - y0
            for dxc, wx in ((x0, 1.0 - wx1), (x0 + 1, wx1)):
                if abs(wx) < 1e-9:
                    continue
                for dyc, wy in ((y0, 1.0 - wy1), (y0 + 1, wy1)):
                    if abs(wy) < 1e-9:
                        continue
                    if abs(dxc) >= W or abs(dyc) > HALO:
                        continue  # fully out of range / beyond halo
                    terms[h][p].append((int(dxc), int(dyc), float(wx * wy)))
    return terms


def _build_smats(terms):
    """Dedupe (dx, alpha) -> [128,128] shift matrices; rewrite terms to
    (s_fwd, s_bwd, dy): out += S_dx @ (V_window * (alpha*S_-dx^T E))."""
    key_to_idx = {}
    mats = []

    def smat(dx, alpha):
        key = (dx, round(alpha, 9))
        if key not in key_to_idx:
            m = np.zeros((P, P), np.float32)
            for j_out in range(W):
                j_in = j_out + dx
                if 0 <= j_in < W:
                    m[j_in, j_out] = alpha
            key_to_idx[key] = len(mats)
            mats.append(m)
        return key_to_idx[key]

    terms2 = [[[] for _ in range(NP)] for _ in range(NH)]
    for h in range(NH):
        for p in range(NP):
            for dx, dy, alpha in terms[h][p]:
                terms2[h][p].append(
                    (smat(dx, 1.0), smat(-dx, alpha), dy))
    return np.stack(mats, 0), terms2


def _np_reference(x, v_w, v_b, aw_w, aw_b, off_w, off_b, proj_w, proj_b, Hh, Ww):
    """Pure-numpy fallback mirroring reference.py (used only if off_w != 0,
    which cannot happen with this problem's setup_inputs)."""
    B, N, C = x.shape
    v = (x @ v_w.T + v_b).reshape(B, N, NH, HD).transpose(0, 2, 1, 3)
    v = v.reshape(B * NH, Hh, Ww, HD)
    mh, mw = np.meshgrid(np.arange(Hh, dtype=x.dtype), np.arange(Ww, dtype=x.dtype),
                         indexing="ij")
    ref = np.stack([mw, mh], -1).reshape(1, N, 1, 2)
    off = (x @ off_w.T + off_b).reshape(B, N, NH, NP, 2).transpose(0, 2, 1, 3, 4)
    off = off.reshape(B * NH, N, NP, 2)
    grid = ref + off
    w = (x @ aw_w.T + aw_b).reshape(B, N, NH, NP).transpose(0, 2, 1, 3)
    w = w.reshape(B * NH, N, NP)
    w = np.exp(w - w.max(-1, keepdims=True))
    w = w / w.sum(-1, keepdims=True)
    G = B * NH
    vf = v.reshape(G, Hh * Ww, HD)
    gx, gy = grid[..., 0], grid[..., 1]
    x0 = np.floor(gx); y0 = np.floor(gy)
    wx1 = gx - x0; wx0 = 1.0 - wx1
    wy1 = gy - y0; wy0 = 1.0 - wy1
    x0i = x0.astype(np.int64); y0i = y0.astype(np.int64)

    def gather(xi, yi):
        valid = (xi >= 0) & (xi < Ww) & (yi >= 0) & (yi < Hh)
        idx = (np.clip(yi, 0, Hh - 1) * Ww + np.clip(xi, 0, Ww - 1))
        g = np.take_along_axis(vf, idx.reshape(G, -1, 1), axis=1)
        return g.reshape(*xi.shape, HD) * valid[..., None]

    samp = ((wy0 * wx0)[..., None] * gather(x0i, y0i)
            + (wy0 * wx1)[..., None] * gather(x0i + 1, y0i)
            + (wy1 * wx0)[..., None] * gather(x0i, y0i + 1)
            + (wy1 * wx1)[..., None] * gather(x0i + 1, y0i + 1))
    out = np.einsum("gnpd,gnp->gnd", samp, w)
    out = out.reshape(B, NH, N, HD).transpose(0, 2, 1, 3).reshape(B, N, C)
    return (out @ proj_w.T + proj_b).astype(np.float32)


def _build_program(terms, n_smats, has_bias=True):
    import concourse.bass as bass
    import concourse.mybir as mybir
    import concourse.tile as tile
    from concourse import bacc

    dt = mybir.dt
    f32 = dt.float32

    fr = dt.float32r if F32R else f32

    nc = bacc.Bacc("TRN2", target_bir_lowering=False, debug=False,
                   num_devices=N_CORES)

    NCH = 256 + NH * NP  # 288: v channels + aw logits per row matmul

    # ---- DRAM I/O ----
    xt_d = nc.dram_tensor("xt_dev", [DIM, TOK_V], fr, kind="ExternalInput")
    ones_d = nc.dram_tensor("ones_dev", [1, TOK_V], fr, kind="ExternalInput")
    wb_d = nc.dram_tensor("wb_cat", [2, P, NCH], fr, kind="ExternalInput")
    bb_d = nc.dram_tensor("bb_cat", [1, NCH], fr, kind="ExternalInput")
    s_d = nc.dram_tensor("s_mats", [n_smats, P, P], fr, kind="ExternalInput")
    pj_d = nc.dram_tensor("proj_t", [2, 2, P, P], fr, kind="ExternalInput")
    pb_d = nc.dram_tensor("projb_t", [2, P], f32, kind="ExternalInput")
    id_d = nc.dram_tensor("ident", [P, P], fr, kind="ExternalInput")
    y0_d = nc.dram_tensor("y0", [P, ROWS_OUT * W], f32, kind="ExternalOutput")
    y1_d = nc.dram_tensor("y1", [P, ROWS_OUT * W], f32, kind="ExternalOutput")
    y_outs = [y0_d, y1_d]

    NG = ROWS_V // 4           # 18 x-DMA groups of 4 rows
    NGO = ROWS_OUT // 4        # 16 groups for phase D

    with tile.TileContext(nc) as tc:
        with (
            tc.tile_pool(name="const", bufs=1) as cpool,
            tc.tile_pool(name="big", bufs=1) as bigpool,
        ):
            # ---- constants ----
            wb_sb = cpool.tile([P, 2, NCH], fr, tag="wb")
            nc.sync.dma_start(wb_sb[:], wb_d.rearrange("kc k f -> k kc f"))
            bb_sb = cpool.tile([1, NCH], fr, tag="bb")
            nc.sync.dma_start(bb_sb[:], bb_d[:])
            s_sb = cpool.tile([P, n_smats, P], fr, tag="smats")
            nc.sync.dma_start(s_sb[:], s_d.rearrange("s k f -> k s f"))
            pj_sb = cpool.tile([P, 2, 2, P], fr, tag="proj")
            nc.sync.dma_start(pj_sb[:], pj_d.rearrange("kc m k f -> k kc m f"))
            pb_sb = cpool.tile([P, 2], f32, tag="projb")
            nc.sync.dma_start(pb_sb[:], pb_d.rearrange("m k -> k m"))
            id_sb = cpool.tile([P, P], fr, tag="ident")
            nc.sync.dma_start(id_sb[:], id_d[:])

            # ---- persistent big tiles ----
            vl_sb = bigpool.tile([P, NH + 1, ROWS_V, HD], fr, tag="V")
            v_sb = vl_sb[:, :NH]
            outs = [bigpool.tile([P, 32, 2, P], fr, tag="OUT", name="out0"),
                    bigpool.tile([P, 32, 2, P], fr, tag="OUT2", name="out1")]
            es = [bigpool.tile([P, NH * NP, 32], fr, tag="E", name="e0"),
                  bigpool.tile([P, NH * NP, 32], fr, tag="E2", name="e1")]

            abc_pools = (
                tc.tile_pool(name="stA", bufs=2),
                tc.tile_pool(name="psA", bufs=2, space="PSUM"),
                tc.tile_pool(name="psC", bufs=2, space="PSUM"),
                tc.tile_pool(name="wt", bufs=1),
                tc.tile_pool(name="stB", bufs=2),
            )
            stA = abc_pools[0].__enter__()
            psA = abc_pools[1].__enter__()
            psC = abc_pools[2].__enter__()
            wtpool = abc_pools[3].__enter__()
            stB = abc_pools[4].__enter__()

            def phase_a(g):
                """x rows 4g..4g+4: v-proj + logits, pixel-major."""
                tok0 = g * 512
                xt_g = [stA.tile([P, 512], fr, tag=f"xt{kc}", bufs=3,
                                 name=f"xtg{kc}") for kc in range(2)]
                for kc in range(2):
                    nc.sync.dma_start(
                        xt_g[kc][:],
                        xt_d[P * kc:P * kc + P, tok0:tok0 + 512])
                if has_bias:
                    ones_g = stA.tile([1, 512], fr, tag="ones")
                    nc.sync.dma_start(ones_g[:], ones_d[:, tok0:tok0 + 512])
                for rl in range(4):
                    rr = 4 * g + rl      # v-row slot
                    a_ps = psA.tile([P, 512], f32, tag="a_ps", bufs=4)
                    for kc in range(2):
                        nc.tensor.matmul(
                            a_ps[:, :NCH],
                            xt_g[kc][:, P * rl:P * rl + P],
                            wb_sb[:, kc, :], start=(kc == 0),
                            stop=(kc == 1 and not has_bias))
                    if has_bias:
                        nc.tensor.matmul(
                            a_ps[:, :NCH], ones_g[:, P * rl:P * rl + P],
                            bb_sb[:], start=False, stop=True)
                    nc.scalar.copy(
                        vl_sb[:, :, rr, :],
                        a_ps[:, :NCH].rearrange("j (h d) -> j h d", h=NH + 1))

            def phase_b(half, heads=None):
                """exp + softmax over points, all heads, rows of `half`."""
                rr = 32 * half
                e_sb = es[half]
                nc.scalar.activation(
                    e_sb[:].rearrange("j hp i -> j i hp"),
                    vl_sb[:, NH, HALO + rr:HALO + rr + 32, :]
                    .rearrange("j i d -> j i d"),
                    mybir.ActivationFunctionType.Exp)
                z = stB.tile([P, NH, 32], f32, tag="z")
                zr = stB.tile([P, NH, 32], fr, tag="zr")
                ev = e_sb[:].rearrange("j (h p) i -> j h p i", p=NP)
                nc.vector.tensor_tensor(z[:], ev[:, :, 0, :], ev[:, :, 1, :],
                                        op=mybir.AluOpType.add)
                nc.vector.tensor_tensor(z[:], z[:], ev[:, :, 2, :],
                                        op=mybir.AluOpType.add)
                nc.vector.tensor_tensor(z[:], z[:], ev[:, :, 3, :],
                                        op=mybir.AluOpType.add)
                with nc.allow_low_precision(reason="fp32r == fp32 bits"):
                    nc.vector.reciprocal(zr[:], z[:])
                for p in range(NP):
                    nc.vector.tensor_tensor(ev[:, :, p, :], ev[:, :, p, :],
                                            zr[:], op=mybir.AluOpType.mult)

            def phase_c(half, heads=None):
                """weight-then-shift: out += S_dx @ (V_win * (S_-dx^T w))."""
                rr = 32 * half
                e_sb = es[half]
                for h in (range(NH) if heads is None else heads):
                    mh, hl = h // 4, h % 4
                    o_ps = psC.tile([P, 32, HD], f32, tag="oacc", bufs=1)
                    n_terms = sum(len(terms[h][p]) for p in range(NP))
                    t_seen = 0
                    for p in range(NP):
                        for (s_fwd, s_bwd, dy) in terms[h][p]:
                            ep_ps = psC.tile([P, 32], f32, tag="ep", bufs=2)
                            nc.tensor.matmul(
                                ep_ps[:], s_sb[:, s_bwd, :],
                                e_sb[:, 4 * h + p, :], start=True, stop=True)
                            ep = wtpool.tile([P, 32], fr, tag="ep_sb",
                                             bufs=2, name="ep")
                            nc.scalar.copy(ep[:], ep_ps[:])
                            m_t = wtpool.tile([P, 32, HD], fr,
                                              tag=f"wt{t_seen % 2}",
                                              bufs=2, name=f"mt{t_seen % 2}")
                            slot0 = rr + dy + HALO
                            nc.vector.tensor_tensor(
                                m_t[:], v_sb[:, h, slot0:slot0 + 32, :],
                                ep[:].unsqueeze(2).broadcast_to([P, 32, HD]),
                                op=mybir.AluOpType.mult)
                            for ch in range(2):
                                nc.tensor.matmul(
                                    o_ps[:, 16 * ch:16 * ch + 16, :]
                                    .rearrange("j i d -> j (i d)"),
                                    s_sb[:, s_fwd, :],
                                    m_t[:, 16 * ch:16 * ch + 16, :]
                                    .rearrange("j i d -> j (i d)"),
                                    start=(t_seen == 0),
                                    stop=(t_seen == n_terms - 1))
                            t_seen += 1
                    nc.scalar.copy(
                        outs[half][:, :, mh, HD * hl:HD * hl + HD],
                        o_ps[:])

            # ---- emission order: A(<40 rows), B0, C0 overlap A(rest), B1, C1
            for g in range(10):
                phase_a(g)
            phase_b(0)
            ctail = list(range(10, NG))
            def phase_d(halfd, gls=None, evac_dve=False):
                """output projection for row groups of half `halfd`."""
                for gl in (range(NGO // 2) if gls is None else gls):
                    g = halfd * (NGO // 2) + gl
                    i0 = 4 * g
                    ot_sb = []
                    for m in range(2):
                        ot_ps = psA.tile([P, 4, P], fr, tag="a_ps",
                                         name=f"ot{m}", bufs=4)
                        for c in range(4):
                            nc.tensor.transpose(
                                ot_ps[:, c, :],
                                outs[halfd][:, i0 - 32 * halfd + c, m, :],
                                id_sb[:])
                        t = stA.tile([P, 512], fr, tag=f"ot{m}", bufs=1)
                        (nc.vector.tensor_copy if evac_dve else nc.scalar.copy)(
                            t[:], ot_ps[:].rearrange("k c f -> k (c f)"))
                        ot_sb.append(t)
                    for mc in range(2):
                        y_ps = psA.tile([P, 512], f32, tag="a_ps",
                                        name=f"yps{mc}", bufs=4)
                        for kc in range(2):
                            nc.tensor.matmul(y_ps[:], pj_sb[:, kc, mc, :],
                                             ot_sb[kc][:],
                                             start=(kc == 0), stop=(kc == 1))
                        ysb = stA.tile([P, 512], f32, tag=f"y{mc}",
                                       name=f"ysb{mc}")
                        if evac_dve:
                            nc.vector.scalar_tensor_tensor(
                                ysb[:], y_ps[:], 1.0,
                                pb_sb[:, mc:mc + 1].to_broadcast([P, 512]),
                                op0=mybir.AluOpType.mult,
                                op1=mybir.AluOpType.add)
                        else:
                            nc.scalar.activation(
                                ysb[:], y_ps[:],
                                mybir.ActivationFunctionType.Identity,
                                bias=pb_sb[:, mc:mc + 1])
                        nc.sync.dma_start(
                            y_outs[mc][:, 512 * g:512 * g + 512], ysb[:])

            PH = do_c = os.environ.get("KERNEL_PHASES", "abcd")
            nc0 = NH if "c" in PH else (1 if "x" in PH else 0)
            ci = 0
            for i, g in enumerate(ctail):
                phase_a(g)
                if ci < nc0:
                    phase_c(0, heads=[ci]); ci += 1
            for h in range(ci, nc0):
                phase_c(0, heads=[h])
            if "c" in PH:
                phase_b(1)
            for h in range(NH):
                if "c" in PH:
                    phase_c(1, heads=[h])
                if "d" in PH:
                    phase_d(0, gls=[h])
            if "d" in PH:
                phase_d(1)
            for pl in reversed(abc_pools):
                pl.__exit__(None, None, None)

    nc.compile()
    return nc

def kernel(x, v_w, v_b, aw_w, aw_b, off_w, off_b, proj_w, proj_b, H=128, W=128,
           **_unused):
    x = np.ascontiguousarray(np.asarray(x, np.float32))
    v_w = np.asarray(v_w, np.float32); v_b = np.asarray(v_b, np.float32)
    aw_w = np.asarray(aw_w, np.float32); aw_b = np.asarray(aw_b, np.float32)
    off_w = np.asarray(off_w, np.float32); off_b = np.asarray(off_b, np.float32)
    proj_w = np.asarray(proj_w, np.float32); proj_b = np.asarray(proj_b, np.float32)

    if np.any(off_w != 0.0) or int(H) != 128 or int(W) != 128:
        # data-dependent offsets or non-128 map: exact host fallback
        return _np_reference(x, v_w, v_b, aw_w, aw_b, off_w, off_b,
                             proj_w, proj_b, int(H), int(W))

    terms = _build_terms(off_b)
    s_mats, terms2 = _build_smats(terms)

    has_bias = bool(np.any(v_b) or np.any(aw_b))
    key = ("prog", s_mats.shape[0], has_bias,
           tuple(tuple(tuple(tl) for tl in th) for th in terms2))
    if key not in _cache:
        _cache[key] = _build_program(terms2, s_mats.shape[0], has_bias)
    nc = _cache[key]

    B = x.shape[0]
    # ---- host prep, shared across cores ----
    NCH = 256 + NH * NP
    wb_cat = np.empty((2, P, NCH), np.float32)
    for kc in range(2):
        wb_cat[kc, :, :256] = v_w[:, P * kc:P * (kc + 1)].T
        wb_cat[kc, :, 256:] = aw_w[:, P * kc:P * (kc + 1)].T
    bb_cat = np.concatenate([v_b, aw_b]).reshape(1, NCH)
    pj_t = np.empty((2, 2, P, P), np.float32)
    for kc in range(2):
        for mc in range(2):
            pj_t[kc, mc] = proj_w[P * mc:P * (mc + 1), P * kc:P * (kc + 1)].T
    pb_t = proj_b.reshape(2, P)
    ident = np.eye(P, dtype=np.float32)
    shared = dict(wb_cat=np.ascontiguousarray(wb_cat),
                  bb_cat=np.ascontiguousarray(bb_cat),
                  s_mats=np.ascontiguousarray(s_mats),
                  proj_t=np.ascontiguousarray(pj_t),
                  projb_t=np.ascontiguousarray(pb_t),
                  ident=ident)

    xr = x.reshape(B, H, W, DIM)
    in_maps = []
    for d in range(N_CORES):
        b, half = d // 2, d % 2
        r0 = ROWS_OUT * half
        x_dev = np.zeros((ROWS_V, W, DIM), np.float32)
        ones = np.zeros((ROWS_V, W), np.float32)
        lo, hi = max(0, r0 - HALO), min(H, r0 + ROWS_OUT + HALO)
        x_dev[lo - (r0 - HALO):hi - (r0 - HALO)] = xr[b, lo:hi]
        ones[lo - (r0 - HALO):hi - (r0 - HALO)] = 1.0
        m = dict(shared)
        m["xt_dev"] = np.ascontiguousarray(x_dev.reshape(TOK_V, DIM).T)
        m["ones_dev"] = ones.reshape(1, TOK_V)
        in_maps.append(m)

    from concourse import bass_utils
    res = bass_utils.run_bass_kernel_spmd(
        nc, in_maps, core_ids=list(range(N_CORES)),
        trace=os.environ.get("KERNEL_TRACE", "0") == "1")
    kernel.last_results = res

    y = np.empty((B, N_TOK, DIM), np.float32)
    for d in range(N_CORES):
        b, half = d // 2, d % 2
        yd = np.concatenate([res.results[d]["y0"], res.results[d]["y1"]], 0)
        y[b, ROWS_OUT * W * half:ROWS_OUT * W * (half + 1), :] = yd.T
    return y



# revision 17
# speedup vs baseline: 1.4865x; 1.4865x over previous
"""Trainium2 Bass kernel for nn_Attention_34840774705279 (sparse/deformable attention).

Math (matches reference.py):
  v   = x @ v_w.T + v_b            -> per-head maps [B*NH, H, W, HD]
  off = x @ off_w.T + off_b        -> off_w is structurally zero, so offsets are
                                      CONSTANT per (head, point); for this problem
                                      they are (+-p or ~1e-16) => integer shifts.
  w   = softmax_p(x @ aw_w.T + aw_b)
  out[i,j] = sum_p w_p[i,j] * v[i+dy_p, j+dx_p]   (zero outside the map)
  y   = out @ proj_w.T + proj_b

Sharding (8 cores, uniform SPMD program):
  core d -> batch b = d//2, row-half r0 = 64*(d%2). Each core computes ALL 8
  heads for its 64 output rows (8192 tokens) using a 4-row halo of v rows
  (host zero-pads x rows outside the image); the host just concatenates.

Device algorithm (per core), bf16 datapath (tolerance is 2e-2; bf16 keeps
DVE in its 2x perf mode and matmuls at 1 cycle/row):
  A. v+logit projection, pixel-major: per image row r, xT chunks are the
     matmul stationary and [v_w.T | aw_w.T] streams, landing [col j, 288ch]
     in PSUM; evacuated (rotating over Scalar/Vector/Pool engines) into the
     d-major VL tile [j, 9 slots, 32 d, 72 rows] as bf16.
  B. softmax over the 4 points, batched across heads (exp on ScalarE,
     adds/reciprocal/normalize on VectorE), per row-half.
  C. sampling + weighting via weight-then-shift identity
        w .* (S_dx @ V_win) == S_dx @ ((S_-dx^T w) .* V_win):
     dx!=0 heads: the tiny bwd matmuls for ALL (head, point) of a half land
     in ONE PSUM tile, evacuated once; VectorE multiplies the V window
     (d-major, so the weight broadcast is on the middle axis and the 2x DVE
     mode stays on) and the 0/1 column-shift matmuls accumulate in PSUM.
     dx==0 heads (identity shift) skip PE entirely: multiply + add tree on
     VectorE writes the OUT tile directly.
  D. output projection: PE transposes OUT (bf16, both channel halves into
     one 2KB PSUM bank) back to channel-major, y^T = proj.T @ OUT^T; host
     transposes y^T back.
  Emission: A(0..8) | B0 E'0 | A(9..17) interleaved with C0 heads | B1 E'1 |
  C1 heads interleaved with D half-0 | D half-1.
"""

import os
import sys
import math

import numpy as np

sys.path.insert(0, "/opt/trn_rl_repo")

P = 128
H = W = 128
NH, NP, HD = 8, 4, 32
DIM = 256
N_TOK = H * W
ROWS_OUT = 64          # output rows per core
HALO = 4
ROWS_V = ROWS_OUT + 2 * HALO   # 72 v-row slots per core
TOK_V = ROWS_V * W             # 9216
N_CORES = 8
NCH = DIM + NH * NP    # 288

_cache = {}


def _build_terms(off_b):
    """Per (h, p): list of (dx, dy, alpha) corner terms from the constant offsets.

    General for any constant offset (bilinear corners); for this problem each
    (h, p) yields exactly one term with alpha ~= 1."""
    ob = np.asarray(off_b, np.float64).reshape(NH, NP, 2)
    terms = [[[] for _ in range(NP)] for _ in range(NH)]
    for h in range(NH):
        for p in range(NP):
            fx, fy = ob[h, p, 0], ob[h, p, 1]
            x0 = math.floor(fx)
            y0 = math.floor(fy)
            wx1 = fx - x0
            wy1 = fy - y0
            for dxc, wx in ((x0, 1.0 - wx1), (x0 + 1, wx1)):
                if abs(wx) < 1e-9:
                    continue
                for dyc, wy in ((y0, 1.0 - wy1), (y0 + 1, wy1)):
                    if abs(wy) < 1e-9:
                        continue
                    if abs(dxc) >= W or abs(dyc) > HALO:
                        continue  # fully out of range / beyond halo
                    terms[h][p].append((int(dxc), int(dyc), float(wx * wy)))
    return terms


def _build_smats(terms):
    """Dedupe (dx, alpha) -> [128,128] shift matrices; rewrite terms to
    (s_fwd, s_bwd, dy): out += S_dx @ (V_window * (alpha*S_-dx^T E))."""
    key_to_idx = {}
    mats = []

    def smat(dx, alpha):
        key = (dx, round(alpha, 9))
        if key not in key_to_idx:
            m = np.zeros((P, P), np.float32)
            for j_out in range(W):
                j_in = j_out + dx
                if 0 <= j_in < W:
                    m[j_in, j_out] = alpha
            key_to_idx[key] = len(mats)
            mats.append(m)
        return key_to_idx[key]

    terms2 = [[[] for _ in range(NP)] for _ in range(NH)]
    for h in range(NH):
        for p in range(NP):
            for dx, dy, alpha in terms[h][p]:
                terms2[h][p].append(
                    (smat(dx, 1.0), smat(-dx, alpha), dy))
    id_idx = key_to_idx.get((0, 1.0))
    return np.stack(mats, 0), terms2, id_idx


def _np_reference(x, v_w, v_b, aw_w, aw_b, off_w, off_b, proj_w, proj_b, Hh, Ww):
    """Pure-numpy fallback mirroring reference.py (used only if off_w != 0,
    which cannot happen with this problem's setup_inputs)."""
    B, N, C = x.shape
    v = (x @ v_w.T + v_b).reshape(B, N, NH, HD).transpose(0, 2, 1, 3)
    v = v.reshape(B * NH, Hh, Ww, HD)
    mh, mw = np.meshgrid(np.arange(Hh, dtype=x.dtype), np.arange(Ww, dtype=x.dtype),
                         indexing="ij")
    ref = np.stack([mw, mh], -1).reshape(1, N, 1, 2)
    off = (x @ off_w.T + off_b).reshape(B, N, NH, NP, 2).transpose(0, 2, 1, 3, 4)
    off = off.reshape(B * NH, N, NP, 2)
    grid = ref + off
    w = (x @ aw_w.T + aw_b).reshape(B, N, NH, NP).transpose(0, 2, 1, 3)
    w = w.reshape(B * NH, N, NP)
    w = np.exp(w - w.max(-1, keepdims=True))
    w = w / w.sum(-1, keepdims=True)
    G = B * NH
    vf = v.reshape(G, Hh * Ww, HD)
    gx, gy = grid[..., 0], grid[..., 1]
    x0 = np.floor(gx); y0 = np.floor(gy)
    wx1 = gx - x0; wx0 = 1.0 - wx1
    wy1 = gy - y0; wy0 = 1.0 - wy1
    x0i = x0.astype(np.int64); y0i = y0.astype(np.int64)

    def gather(xi, yi):
        valid = (xi >= 0) & (xi < Ww) & (yi >= 0) & (yi < Hh)
        idx = (np.clip(yi, 0, Hh - 1) * Ww + np.clip(xi, 0, Ww - 1))
        g = np.take_along_axis(vf, idx.reshape(G, -1, 1), axis=1)
        return g.reshape(*xi.shape, HD) * valid[..., None]

    samp = ((wy0 * wx0)[..., None] * gather(x0i, y0i)
            + (wy0 * wx1)[..., None] * gather(x0i + 1, y0i)
            + (wy1 * wx0)[..., None] * gather(x0i, y0i + 1)
            + (wy1 * wx1)[..., None] * gather(x0i + 1, y0i + 1))
    out = np.einsum("gnpd,gnp->gnd", samp, w)
    out = out.reshape(B, NH, N, HD).transpose(0, 2, 1, 3).reshape(B, N, C)
    return (out @ proj_w.T + proj_b).astype(np.float32)


def _classify_heads(terms, id_idx):
    """Heads whose every point is a single identity-column-shift term can be
    computed entirely on VectorE (no PE shift matmuls)."""
    dve_heads, mm_heads = [], []
    for h in range(NH):
        ok = id_idx is not None and all(
            len(terms[h][p]) == 1
            and terms[h][p][0][0] == id_idx and terms[h][p][0][1] == id_idx
            for p in range(NP))
        (dve_heads if ok else mm_heads).append(h)
    return dve_heads, mm_heads


def _build_program(terms, n_smats, has_bias, has_pbias):
    import concourse.bass as bass
    import concourse.mybir as mybir
    import concourse.tile as tile
    from concourse import bacc

    dt = mybir.dt
    f32 = dt.float32
    bf16 = dt.bfloat16

    nc = bacc.Bacc("TRN2", target_bir_lowering=False, debug=False,
                   num_devices=N_CORES)

    _, _, id_idx_probe = None, None, None
    # id_idx is passed in via terms' construction; recompute head classes here
    # from the structural property instead (identity == s_fwd == s_bwd with a
    # dy-only shift). The caller passes id_idx through `terms` closure below.
    id_idx = _build_program._id_idx
    dve_heads, mm_heads = _classify_heads(terms, id_idx)
    # E' slot per (h, p, term_index) for mm heads
    ep_slot = {}
    n_slots = 0
    for h in mm_heads:
        for p in range(NP):
            for t in range(len(terms[h][p])):
                ep_slot[(h, p, t)] = n_slots
                n_slots += 1
    assert n_slots <= 32, "E' batch exceeds one PSUM pair; add chunking"

    NG = ROWS_V // 4           # 18 x-DMA groups of 4 rows
    NGO = ROWS_OUT // 4        # 16 groups for phase D

    # ---- DRAM I/O ----
    # consts blob columns: wb (2*NCH) | smats (n_smats*P) | pj (4*P) | id (P)
    CB = 2 * NCH + n_smats * P + 4 * P + P
    xt_d = nc.dram_tensor("xt_dev", [DIM, TOK_V], bf16, kind="ExternalInput")
    cb_d = nc.dram_tensor("consts", [P, CB], bf16, kind="ExternalInput")
    if has_pbias:
        pb_d = nc.dram_tensor("projb_t", [2, P], f32, kind="ExternalInput")
    if has_bias:
        ones_d = nc.dram_tensor("ones_dev", [1, TOK_V], bf16, kind="ExternalInput")
        bb_d = nc.dram_tensor("bb_cat", [1, NCH], bf16, kind="ExternalInput")
    y0_d = nc.dram_tensor("y0", [P, ROWS_OUT * W], f32, kind="ExternalOutput")
    y1_d = nc.dram_tensor("y1", [P, ROWS_OUT * W], f32, kind="ExternalOutput")
    y_outs = [y0_d, y1_d]

    with tile.TileContext(nc) as tc:
        with (
            tc.tile_pool(name="const", bufs=1) as cpool,
            tc.tile_pool(name="big", bufs=1) as bigpool,
            tc.tile_pool(name="stA", bufs=2) as stA,
            tc.tile_pool(name="wt", bufs=1) as wtpool,
            tc.tile_pool(name="stB", bufs=2) as stB,
        ):
            # ---- x^T first chunk goes out before the consts blob ----
            xt_sb = bigpool.tile([P, 2, TOK_V], bf16, tag="XT")
            XT_CHUNKS = [(0, 1), (1, 3), (3, 6), (6, 10), (10, NG)]
            g0, g1 = XT_CHUNKS[0]
            for kc in range(2):
                nc.sync.dma_start(
                    xt_sb[:, kc, 512 * g0:512 * g1],
                    xt_d[P * kc:P * kc + P, 512 * g0:512 * g1])
            cb_sb = cpool.tile([P, CB], bf16, tag="cblob")
            nc.sync.dma_start(cb_sb[:], cb_d[:])
            o0 = 0
            wb_sb = cb_sb[:, o0:o0 + 2 * NCH].rearrange(
                "j (kc f) -> j kc f", kc=2)
            o0 += 2 * NCH
            s_sb = cb_sb[:, o0:o0 + n_smats * P].rearrange(
                "j (s f) -> j s f", s=n_smats)
            o0 += n_smats * P
            pj_sb = cb_sb[:, o0:o0 + 4 * P].rearrange(
                "j (kc m f) -> j kc m f", kc=2, m=2)
            o0 += 4 * P
            id_sb = cb_sb[:, o0:o0 + P]
            if has_pbias:
                pb_sb = cpool.tile([P, 2], f32, tag="projb")
                nc.sync.dma_start(pb_sb[:], pb_d.rearrange("m k -> k m"))
            if has_bias:
                bb_sb = cpool.tile([1, NCH], bf16, tag="bb")
                nc.sync.dma_start(bb_sb[:], bb_d[:])

            # ---- rest of x^T in large chunked DMAs ----
            for (g0, g1) in XT_CHUNKS[1:]:
                for kc in range(2):
                    nc.sync.dma_start(
                        xt_sb[:, kc, 512 * g0:512 * g1],
                        xt_d[P * kc:P * kc + P, 512 * g0:512 * g1])
            if has_bias:
                ones_sb = bigpool.tile([1, TOK_V], bf16, tag="ONES")
                nc.sync.dma_start(ones_sb[:], ones_d[:])

            # ---- persistent big tiles (all bf16, d-major) ----
            # vl: [j, slot(8 heads + logits), d, row]
            vl_sb = bigpool.tile([P, NH + 1, HD, ROWS_V], bf16, tag="V")
            v_sb = vl_sb[:, :NH]
            # outs[half]: [j, mc, hl, d, i]  (channel ch = mc*128+hl*32+d)
            outs = [bigpool.tile([P, 2, 4, HD, 32], bf16, tag="OUT", name="out0"),
                    bigpool.tile([P, 2, 4, HD, 32], bf16, tag="OUT2", name="out1")]
            es = [bigpool.tile([P, NH * NP, 32], bf16, tag="E", name="e0"),
                  bigpool.tile([P, NH * NP, 32], bf16, tag="E2", name="e1")]
            eps = [bigpool.tile([P, max(n_slots, 1), 32], bf16, tag="EP",
                                name="ep0"),
                   bigpool.tile([P, max(n_slots, 1), 32], bf16, tag="EP2",
                                name="ep1")]

            def phase_a(g, evac_eng):
                """x rows 4g..4g+4: v-proj + logits, pixel-major.
                PSUM is evacuated in row pairs to halve the fixed cost."""
                tok0 = g * 512
                for pr in range(2):
                    # rows padded to 512 so each matmul output stays inside
                    # one 2KB PSUM bank
                    a_ps = psA.tile([P, 2, 512], f32, tag="a_ps", bufs=3)
                    for rh in range(2):
                        rl = 2 * pr + rh
                        for kc in range(2):
                            nc.tensor.matmul(
                                a_ps[:, rh, :NCH],
                                xt_sb[:, kc, tok0 + P * rl:tok0 + P * rl + P],
                                wb_sb[:, kc, :], start=(kc == 0),
                                stop=(kc == 1 and not has_bias))
                        if has_bias:
                            nc.tensor.matmul(
                                a_ps[:, rh, :NCH],
                                ones_sb[:, tok0 + P * rl:tok0 + P * rl + P],
                                bb_sb[:], start=False, stop=True)
                    rr = 4 * g + 2 * pr
                    eng = evac_eng(rr)
                    dst = vl_sb[:, :, :, rr:rr + 2].rearrange(
                        "j s d r -> j r (s d)")
                    if eng is nc.scalar:
                        nc.scalar.copy(dst, a_ps[:, :, :NCH])
                    else:
                        eng.tensor_copy(dst, a_ps[:, :, :NCH])

            def phase_b(half, eng=None):
                """exp + softmax over points, all heads, rows of `half`."""
                rr = 32 * half
                e_sb = es[half]
                eng = eng or nc.vector
                nc.scalar.activation(
                    e_sb[:],
                    vl_sb[:, NH, :, HALO + rr:HALO + rr + 32],
                    mybir.ActivationFunctionType.Exp)
                ev = e_sb[:].rearrange("j (h p) i -> j h p i", p=NP)
                z = stB.tile([P, NH, 32], bf16, tag="z")
                zr = stB.tile([P, NH, 32], bf16, tag="zr")
                with nc.allow_low_precision(reason="softmax denom in bf16; "
                                            "tolerance is 2e-2"):
                    eng.tensor_tensor(z[:], ev[:, :, 0, :], ev[:, :, 1, :],
                                      op=mybir.AluOpType.add)
                    eng.tensor_tensor(z[:], z[:], ev[:, :, 2, :],
                                      op=mybir.AluOpType.add)
                    eng.tensor_tensor(z[:], z[:], ev[:, :, 3, :],
                                      op=mybir.AluOpType.add)
                    nc.vector.reciprocal(zr[:], z[:])
                    for p in range(NP):
                        eng.tensor_tensor(ev[:, :, p, :], ev[:, :, p, :],
                                          zr[:], op=mybir.AluOpType.mult)

            def phase_e(half):
                """Batched E' = alpha * S_-dx^T E for all mm-head terms.
                Borrows the oacc PSUM rotation (same bank pair)."""
                if n_slots == 0:
                    return
                e_sb = es[half]
                ep_ps = psO.tile([P, 32, 32], f32, tag="oacc", bufs=1,
                                 name="ep_ps")
                for h in mm_heads:
                    for p in range(NP):
                        for t, (s_fwd, s_bwd, dy) in enumerate(terms[h][p]):
                            nc.tensor.matmul(
                                ep_ps[:, ep_slot[(h, p, t)], :],
                                s_sb[:, s_bwd, :],
                                e_sb[:, NP * h + p, :], start=True, stop=True)
                nc.scalar.copy(eps[half][:, :n_slots, :], ep_ps[:, :n_slots, :])

            def phase_c(half, h, out_evac_eng):
                """Per-head sampling: weight-then-shift (mm) or pure-DVE."""
                rr = 32 * half
                e_sb = es[half]
                mh, hl = h // 4, h % 4
                dst = outs[half][:, mh, hl, :, :]
                if h in dve_heads:
                    # out = sum_p E_p .* V_win(dy_p): multiplies on VectorE,
                    # add tree on Pool (SBUF-only ops are legal there)
                    ms = []
                    for p in range(NP):
                        dy = terms[h][p][0][2]
                        slot0 = rr + dy + HALO
                        m_t = wtpool.tile([P, HD, 32], bf16,
                                          tag=f"dv{p}", bufs=2,
                                          name=f"dv{p}")
                        with nc.allow_low_precision(reason="bf16 sampling"):
                            nc.vector.tensor_tensor(
                                m_t[:], v_sb[:, h, :, slot0:slot0 + 32],
                                e_sb[:, NP * h + p, :].unsqueeze(1)
                                .broadcast_to([P, HD, 32]),
                                op=mybir.AluOpType.mult)
                        ms.append(m_t)
                    a1 = wtpool.tile([P, HD, 32], bf16, tag="dva",
                                     bufs=2, name="dva")
                    a2 = wtpool.tile([P, HD, 32], bf16, tag="dvb",
                                     bufs=2, name="dvb")
                    with nc.allow_low_precision(reason="bf16 acc"):
                        nc.gpsimd.tensor_tensor(a1[:], ms[0][:], ms[1][:],
                                                op=mybir.AluOpType.add)
                        nc.gpsimd.tensor_tensor(a2[:], ms[2][:], ms[3][:],
                                                op=mybir.AluOpType.add)
                        nc.gpsimd.tensor_tensor(dst, a1[:], a2[:],
                                                op=mybir.AluOpType.add)
                    return
                # mm path
                n_terms = sum(len(terms[h][p]) for p in range(NP))
                if n_terms == 0:
                    nc.gpsimd.memset(dst, 0.0)
                    return
                o_ps = psO.tile([P, HD, 32], f32, tag="oacc", bufs=1)
                t_seen = 0
                for p in range(NP):
                    for t, (s_fwd, s_bwd, dy) in enumerate(terms[h][p]):
                        slot0 = rr + dy + HALO
                        m_t = wtpool.tile([P, HD, 32], bf16,
                                          tag=f"wt{t_seen % 2}", bufs=2,
                                          name=f"mt{t_seen % 2}")
                        with nc.allow_low_precision(reason="bf16 sampling"):
                            nc.vector.tensor_tensor(
                                m_t[:], v_sb[:, h, :, slot0:slot0 + 32],
                                eps[half][:, ep_slot[(h, p, t)], :]
                                .unsqueeze(1).broadcast_to([P, HD, 32]),
                                op=mybir.AluOpType.mult)
                        for ch in range(2):
                            nc.tensor.matmul(
                                o_ps[:, 16 * ch:16 * ch + 16, :]
                                .rearrange("j d i -> j (d i)"),
                                s_sb[:, s_fwd, :],
                                m_t[:, 16 * ch:16 * ch + 16, :]
                                .rearrange("j d i -> j (d i)"),
                                start=(t_seen == 0),
                                stop=(t_seen == n_terms - 1))
                        t_seen += 1
                eng = out_evac_eng
                if eng is nc.scalar:
                    nc.scalar.copy(dst, o_ps[:])
                else:
                    eng.tensor_copy(dst, o_ps[:])

            y_chunks = {}

            def phase_d(halfd, gl, ot_eng, y_eng):
                """output projection for one 4-row group of half `halfd`.
                y rows collect into a chunk tile, DMA'd out per 4 groups."""
                g = halfd * (NGO // 2) + gl
                i0 = 4 * gl
                ot_ps = psD.tile([P, 2, 4, P], bf16, tag="ot", bufs=2)
                for kc in range(2):
                    for c in range(4):
                        nc.tensor.transpose(
                            ot_ps[:, kc, c, :],
                            outs[halfd][:, kc, :, :, i0 + c]
                            .rearrange("j hl d -> j (hl d)"),
                            id_sb[:])
                ot_sb = stA.tile([P, 2, 4, P], bf16, tag="ot_sb", bufs=2)
                if ot_eng is nc.scalar:
                    nc.scalar.copy(ot_sb[:], ot_ps[:])
                else:
                    ot_eng.tensor_copy(ot_sb[:], ot_ps[:])
                y_ps = psD.tile([P, 2, 512], f32, tag="yps", bufs=2)
                for mc in range(2):
                    for kc in range(2):
                        nc.tensor.matmul(
                            y_ps[:, mc, :], pj_sb[:, kc, mc, :],
                            ot_sb[:, kc].rearrange("j c f -> j (c f)"),
                            start=(kc == 0), stop=(kc == 1))
                single = g >= 2 * NGO - 2
                if g % 2 == 0 or single:
                    y_chunks[0] = stA.tile([P, 2, 2, 512], f32, tag="ysb",
                                           name="ysb_ch", bufs=2)
                ysb_ch = y_chunks[0]
                ci = 0 if single else g % 2
                dst = ysb_ch[:, :, ci, :]
                if has_pbias:
                    for mc in range(2):
                        nc.scalar.activation(
                            dst[:, mc, :], y_ps[:, mc, :],
                            mybir.ActivationFunctionType.Identity,
                            bias=pb_sb[:, mc:mc + 1])
                elif y_eng is nc.scalar:
                    nc.scalar.copy(dst, y_ps[:])
                else:
                    y_eng.tensor_copy(dst, y_ps[:])
                if single:
                    for mc in range(2):
                        nc.sync.dma_start(
                            y_outs[mc][:, 512 * g:512 * (g + 1)],
                            ysb_ch[:, mc, 0, :])
                elif ci == 1:
                    for mc in range(2):
                        nc.sync.dma_start(
                            y_outs[mc][:, 1024 * (g // 2):1024 * (g // 2 + 1)],
                            ysb_ch[:, mc, :, :].rearrange("j c f -> j (c f)"))

            # ================= emission =================
            # A-evac engine: seg1 alternates Vector/Scalar; A-tail all Scalar
            seg1_rot = [nc.vector, nc.scalar]

            def a_evac_seg1(rr):
                return seg1_rot[(rr // 2) % 2]

            def a_evac_tail(rr):
                return nc.scalar

            cmA = tc.tile_pool(name="psA", bufs=1, space="PSUM")
            psA = cmA.__enter__()
            for g in range(9):
                phase_a(g, a_evac_seg1)
            phase_b(0)
            cmO = tc.tile_pool(name="psO", bufs=1, space="PSUM")
            psO = cmO.__enter__()
            phase_e(0)
            order0 = dve_heads + mm_heads
            ci = 0
            for g in range(9, NG):
                phase_a(g, a_evac_tail)
                if ci < NH:
                    phase_c(0, order0[ci], nc.scalar)
                    ci += 1
            while ci < NH:
                phase_c(0, order0[ci], nc.scalar)
                ci += 1
            phase_b(1)
            phase_e(1)
            cmO.__exit__(None, None, None)
            cmA.__exit__(None, None, None)

            cmO = tc.tile_pool(name="psO2", bufs=1, space="PSUM")
            psO = cmO.__enter__()
            cmD = tc.tile_pool(name="psD", bufs=1, space="PSUM")
            psD = cmD.__enter__()
            order1 = dve_heads + mm_heads
            for i in range(NH):
                phase_d(0, i, nc.vector, nc.scalar)
                phase_c(1, order1[i], nc.scalar)
            for gl in range(NGO // 2):
                phase_d(1, gl, nc.vector, nc.scalar)
            cmD.__exit__(None, None, None)
            cmO.__exit__(None, None, None)

    nc.compile()
    return nc


def kernel(x, v_w, v_b, aw_w, aw_b, off_w, off_b, proj_w, proj_b, H=128, W=128,
           **_unused):
    import ml_dtypes
    bf16 = ml_dtypes.bfloat16

    x = np.ascontiguousarray(np.asarray(x, np.float32))
    v_w = np.asarray(v_w, np.float32); v_b = np.asarray(v_b, np.float32)
    aw_w = np.asarray(aw_w, np.float32); aw_b = np.asarray(aw_b, np.float32)
    off_w = np.asarray(off_w, np.float32); off_b = np.asarray(off_b, np.float32)
    proj_w = np.asarray(proj_w, np.float32); proj_b = np.asarray(proj_b, np.float32)

    if np.any(off_w != 0.0) or int(H) != 128 or int(W) != 128:
        # data-dependent offsets or non-128 map: exact host fallback
        return _np_reference(x, v_w, v_b, aw_w, aw_b, off_w, off_b,
                             proj_w, proj_b, int(H), int(W))

    terms = _build_terms(off_b)
    s_mats, terms2, id_idx = _build_smats(terms)

    has_bias = bool(np.any(v_b) or np.any(aw_b))
    has_pbias = bool(np.any(proj_b))
    key = ("prog", s_mats.shape[0], has_bias, has_pbias, id_idx,
           tuple(tuple(tuple(tl) for tl in th) for th in terms2))
    if key not in _cache:
        _build_program._id_idx = id_idx
        _cache[key] = _build_program(terms2, s_mats.shape[0], has_bias,
                                     has_pbias)
    nc = _cache[key]

    B = x.shape[0]
    # ---- host prep, shared across cores ----
    wb_cat = np.empty((2, P, NCH), np.float32)
    for kc in range(2):
        wb_cat[kc, :, :256] = v_w[:, P * kc:P * (kc + 1)].T
        wb_cat[kc, :, 256:] = aw_w[:, P * kc:P * (kc + 1)].T
    pj_t = np.empty((2, 2, P, P), np.float32)
    for kc in range(2):
        for mc in range(2):
            pj_t[kc, mc] = proj_w[P * mc:P * (mc + 1), P * kc:P * (kc + 1)].T
    pb_t = proj_b.reshape(2, P)
    ident = np.eye(P, dtype=np.float32)
    blob = np.concatenate(
        [wb_cat[0], wb_cat[1]]
        + [s_mats[s] for s in range(s_mats.shape[0])]
        + [pj_t[kc, mc] for kc in range(2) for mc in range(2)]
        + [ident], axis=1)
    shared = dict(consts=np.ascontiguousarray(blob.astype(bf16)))
    if has_pbias:
        shared["projb_t"] = np.ascontiguousarray(pb_t)
    if has_bias:
        bb_cat = np.concatenate([v_b, aw_b]).reshape(1, NCH)
        shared["bb_cat"] = np.ascontiguousarray(bb_cat.astype(bf16))

    xr = x.reshape(B, H, W, DIM)
    in_maps = []
    for d in range(N_CORES):
        b, half = d // 2, d % 2
        r0 = ROWS_OUT * half
        x_dev = np.zeros((ROWS_V, W, DIM), np.float32)
        lo, hi = max(0, r0 - HALO), min(H, r0 + ROWS_OUT + HALO)
        x_dev[lo - (r0 - HALO):hi - (r0 - HALO)] = xr[b, lo:hi]
        m = dict(shared)
        m["xt_dev"] = np.ascontiguousarray(
            x_dev.reshape(TOK_V, DIM).T.astype(bf16))
        if has_bias:
            ones = np.zeros((ROWS_V, W), np.float32)
            ones[lo - (r0 - HALO):hi - (r0 - HALO)] = 1.0
            m["ones_dev"] = ones.reshape(1, TOK_V).astype(bf16)
        in_maps.append(m)

    from concourse import bass_utils
    res = bass_utils.run_bass_kernel_spmd(
        nc, in_maps, core_ids=list(range(N_CORES)),
        trace=os.environ.get("KERNEL_TRACE", "0") == "1")
    kernel.last_results = res

    y = np.empty((B, N_TOK, DIM), np.float32)
    for d in range(N_CORES):
        b, half = d // 2, d % 2
        yd = np.concatenate([res.results[d]["y0"], res.results[d]["y1"]], 0)
        y[b, ROWS_OUT * W * half:ROWS_OUT * W * (half + 1), :] = yd.T
    return y
